# revision 19
# baseline (speedup 1.0000x reference)
"""Trainium2 Bass kernel for nn_KPModel (AFT encoder + AFT decoder + softmax).

v2 strategy (vs v1's per-instance serial program):
- 8 instances per core processed in LAYER LOCKSTEP: each phase (qkv,
  aft, norm+ff, ...) runs across all 8 instances before moving on.  This
  batches scalar-engine activations by table set (exp/tanh/relu vs sqrt)
  so ACT_TABLE_LOADs drop from ~21/instance to ~2/layer, and gives every
  engine 8 independent dependency chains to pipeline.
- bf16 operands for all matmuls and most elementwise ops (PSUM stays
  f32): 2-4x DVE throughput, FWL weight loads, half SBUF footprint.
- sigmoid(x) -> 0.5*(1+tanh(x/2)): tanh lives in the same activation
  table set as exp/relu, eliminating the sigmoid table set entirely.
- PSUM discipline: 4 tags x 2 bufs x [128,512]f32 = exactly 8 banks.
- decoder query via rank-1 trick: q = dWq_m^T gmean (+) dwq_last (x) cap,
  computed as a 1-col matmul + a K=1 rank-1 matmul + activation bias.
"""

import numpy as np
import ml_dtypes

try:
    import concourse.bass as bass  # noqa: F401
except ImportError:  # pragma: no cover
    import sys

    sys.path.insert(0, "/opt/trn_rl_repo")
    import concourse.bass as bass

from contextlib import ExitStack

from concourse import bacc
import concourse.mybir as mybir
from concourse.bass_utils import run_bass_kernel_spmd
from concourse.tile import TileContext

F32 = mybir.dt.float32
F32R = mybir.dt.float32r
BF = mybir.dt.bfloat16
AF = mybir.ActivationFunctionType
ALU = mybir.AluOpType
AX = mybir.AxisListType
I32 = mybir.dt.int32
RSQRT_MAGIC = 0x5F3759DF

B, P, N, E, F, L = 64, 512, 512, 128, 512, 6
SQRT_E = 11.313708498984761
CLIP = 10.0
EPS = 1e-5
NCORES = 8
BI = B // NCORES
NT = N // 128
PT = P // 128
FT = F // 128

# which engine runs the relu for each of the 4 FF tiles ("s"=scalar, "v"=DVE)
RELU_ENG = ("s", "s", "s", "s")
# how many instances run the inorm applies on the scalar engine (rest: DVE)
N_A2_SC = 8
N_H_SC = 8


def _build(nc, meta):
    ls = meta["ls"]
    s_enc = [ls * a for a in meta["aft_alpha"]]
    alphas_equal = all(abs(s - s_enc[0]) < 1e-12 for s in s_enc)
    assert alphas_equal, "host pre-exp requires equal aft alphas"
    s_dec = ls * meta["dec_aft_alpha"]
    s_probs = ls * meta["probs_alpha"]
    use_ninf = meta["use_ninf"]
    n1w_ones, n1b_zero = meta["n1w_ones"], meta["n1b_zero"]
    n2w_ones, n2b_zero = meta["n2w_ones"], meta["n2b_zero"]
    b1_zero, b2_zero = meta["b1_zero"], meta["b2_zero"]
    embb_zero = meta["embb_zero"]

    # ---------------- DRAM I/O ----------------
    d_data = nc.dram_tensor("data_t", [BI, 2, N], F32R, kind="ExternalInput")
    d_distT = nc.dram_tensor("dist_t", [BI, N, N], BF, kind="ExternalInput")
    d_cd = nc.dram_tensor("cur_dist", [BI, P, N], BF, kind="ExternalInput")
    d_cdT = nc.dram_tensor("cur_dist_t", [BI, N, P], BF, kind="ExternalInput")
    d_cap = nc.dram_tensor("capacity", [BI, P], F32R, kind="ExternalInput")
    if use_ninf:
        d_ninf = nc.dram_tensor("ninf", [BI, P, N], F32, kind="ExternalInput")
    d_embw = nc.dram_tensor("emb_w", [2, E], F32R, kind="ExternalInput")
    d_embb = None
    if not embb_zero:
        d_embb = nc.dram_tensor("emb_b", [E, 1], F32, kind="ExternalInput")
    d_wq = nc.dram_tensor("wq_p", [E, L * E], BF, kind="ExternalInput")
    d_wkv = nc.dram_tensor("wkv_p", [E, L * 2 * E], BF, kind="ExternalInput")
    d_w1 = nc.dram_tensor("w1_p", [E, L * F], BF, kind="ExternalInput")
    d_w2 = nc.dram_tensor("w2_p", [128, L * FT * E], BF, kind="ExternalInput")
    d_n1w = d_n1b = d_n2w = d_n2b = d_b1 = d_b2 = None
    if not n1w_ones:
        d_n1w = nc.dram_tensor("n1wp", [E, L], F32, kind="ExternalInput")
    if not n1b_zero:
        d_n1b = nc.dram_tensor("n1bp", [E, L], F32, kind="ExternalInput")
    if not n2w_ones:
        d_n2w = nc.dram_tensor("n2wp", [E, L], F32, kind="ExternalInput")
    if not n2b_zero:
        d_n2b = nc.dram_tensor("n2bp", [E, L], F32, kind="ExternalInput")
    if not b1_zero:
        d_b1 = nc.dram_tensor("b1p", [128, L * FT], F32, kind="ExternalInput")
    if not b2_zero:
        d_b2 = nc.dram_tensor("b2p", [E, L], F32, kind="ExternalInput")
    d_dwqm = nc.dram_tensor("dwq_main", [E, E], BF, kind="ExternalInput")
    d_ident = nc.dram_tensor("ident", [128, 128], BF, kind="ExternalInput")
    d_dwql = nc.dram_tensor("dwq_last", [1, E], F32R, kind="ExternalInput")
    d_dkv = nc.dram_tensor("dkv", [E, 2 * E], BF, kind="ExternalInput")
    d_out = nc.dram_tensor("out", [BI, P, N], F32, kind="ExternalOutput")

    with TileContext(nc) as tc, ExitStack() as es, \
            nc.allow_low_precision("bf16 activations by design; fp32 psum"):
        cw = es.enter_context(tc.tile_pool(name="cw", bufs=1))
        pers = es.enter_context(tc.tile_pool(name="pers", bufs=1))
        wk2 = es.enter_context(tc.tile_pool(name="wk2", bufs=2))
        wk3 = es.enter_context(tc.tile_pool(name="wk3", bufs=3))
        wk4 = es.enter_context(tc.tile_pool(name="wk4", bufs=4))
        wk6 = es.enter_context(tc.tile_pool(name="wk6", bufs=6))
        pp = es.enter_context(tc.tile_pool(name="pp", bufs=2, space="PSUM"))
        pq = es.enter_context(tc.tile_pool(name="pq", bufs=2, space="PSUM"))
        pn = es.enter_context(tc.tile_pool(name="pn", bufs=2, space="PSUM"))

        # ---------------- weights ----------------
        t_embw = cw.tile([2, E], F32R)
        nc.sync.dma_start(t_embw, d_embw[:])
        t_embb = None
        if not embb_zero:
            t_embb = cw.tile([E, 1], F32)
            nc.sync.dma_start(t_embb, d_embb[:])
        t_wq = cw.tile([E, L * E], BF)
        nc.sync.dma_start(t_wq, d_wq[:])
        t_wkv = cw.tile([E, L * 2 * E], BF)
        nc.sync.dma_start(t_wkv, d_wkv[:])
        t_w1 = cw.tile([E, L * F], BF)
        t_w2 = cw.tile([128, L * FT * E], BF)

        def _maybe(dram, shape):
            if dram is None:
                return None
            t = cw.tile(shape, F32)
            nc.sync.dma_start(t, dram[:])
            return t

        t_n1w = _maybe(d_n1w, [E, L])
        t_n1b = _maybe(d_n1b, [E, L])
        t_n2w = _maybe(d_n2w, [E, L])
        t_n2b = _maybe(d_n2b, [E, L])
        t_b1 = _maybe(d_b1, [128, L * FT])
        t_b2 = _maybe(d_b2, [E, L])
        t_dwqm = cw.tile([E, E], BF)
        nc.sync.dma_start(t_dwqm, d_dwqm[:])
        t_ident = cw.tile([128, 128], BF)
        nc.sync.dma_start(t_ident, d_ident[:])
        t_dwql = cw.tile([1, E], F32R)
        nc.sync.dma_start(t_dwql, d_dwql[:])
        t_dkv = cw.tile([E, 2 * E], BF)
        nc.sync.dma_start(t_dkv, d_dkv[:])
        t_eps = cw.tile([128, 1], F32)
        nc.vector.memset(t_eps, EPS)

        # per-instance persistent tiles (tag-addressed, bufs=1)
        adT = {}
        xs = {}

        def load_adT(i, scale):
            """DMA the host-preexponentiated adT slab for instance i."""
            adT[i] = pers.tile([128, NT * N], BF, tag=f"adT{i}", name=f"adT{i}")
            for t in range(NT):
                nc.sync.dma_start(adT[i][:, t * N:(t + 1) * N],
                                  d_distT[i, t * 128:(t + 1) * 128, :])

        def emit_phase0(i):
            """Embedding then adT for instance i (fused into layer 0 phase A).

            dat/emb first so the PE's first matmul only waits on two tiny
            DMAs, not the 1MB dist slab."""
            dat = wk2.tile([2, N], F32R, tag="dat")
            nc.sync.dma_start(dat, d_data[i])
            ps_e = pq.tile([128, 512], F32, tag="q")
            nc.tensor.matmul(ps_e, lhsT=t_embw, rhs=dat, start=True, stop=True)
            xs[i] = pers.tile([E, N], BF, tag=f"x{i}", name=f"x{i}")
            if embb_zero:
                nc.scalar.activation(xs[i], ps_e, AF.Copy)
            else:
                nc.scalar.activation(xs[i], ps_e, AF.Identity,
                                     bias=t_embb[:, 0:1])
            if alphas_equal:
                load_adT(i, s_enc[0])

        # ---------------- encoder layers ----------------
        eks, ekvs, ths, prehs, prexs = {}, {}, {}, {}, {}
        hs = {}

        def batched_rsqrt(mvall, tagp, lo, hi):
            """rs[:, i-lo] = 1/sqrt(var_i + eps) for instances lo..hi-1.

            Quake bit-trick seed + 2 Newton iterations, all on DVE int/f32
            ALU ops, so the scalar engine never needs the sqrt table set.
            """
            nb = hi - lo
            ve = wk2.tile([128, nb], F32, tag=f"rq_ve{lo}", name=f"ve_{tagp}")
            nc.vector.tensor_scalar(ve, mvall[:, 8 + lo:8 + hi], scalar1=EPS,
                                    scalar2=None, op0=ALU.add)
            tb = wk2.tile([128, nb], I32, tag=f"rq_tb{lo}", name=f"tb_{tagp}")
            nc.vector.tensor_scalar(tb, ve.bitcast(I32), scalar1=1,
                                    scalar2=None, op0=ALU.arith_shift_right)
            sd = wk2.tile([128, nb], I32, tag=f"rq_sd{lo}", name=f"sd_{tagp}")
            nc.vector.tensor_scalar(sd, tb, scalar1=-1, scalar2=RSQRT_MAGIC,
                                    op0=ALU.mult, op1=ALU.add)
            y = sd.bitcast(F32)
            for it in range(1):
                a = wk2.tile([128, nb], I32 if False else F32,
                             tag=f"rq_a{it}{lo}", name=f"a{it}_{tagp}")
                nc.vector.tensor_tensor(a, y, y, op=ALU.mult)
                b = wk2.tile([128, nb], F32, tag=f"rq_b{it}{lo}",
                             name=f"b{it}_{tagp}")
                nc.vector.tensor_tensor(b, a, ve, op=ALU.mult)
                c = wk2.tile([128, nb], F32, tag=f"rq_c{it}{lo}",
                             name=f"c{it}_{tagp}")
                nc.vector.tensor_scalar(c, b, scalar1=-0.5, scalar2=1.5,
                                        op0=ALU.mult, op1=ALU.add)
                y2 = wk2.tile([128, nb], F32, tag=f"rq_y{it}{lo}",
                              name=f"y{it}_{tagp}")
                nc.vector.tensor_tensor(y2, y, c, op=ALU.mult)
                y = y2
            return y

        prev = {}

        def emit_apply2(i):
            if i < 4:
                rs2c = prev["rs2a"][:, i:i + 1]
                nm2c = prev["nm2a"][:, i:i + 1]
            else:
                rs2c = prev["rs2b"][:, i - 4:i - 3]
                nm2c = prev["nm2b"][:, i - 4:i - 3]
            xs[i] = pers.tile([E, N], BF, tag=f"x{i}", name=f"x{i}")
            if i < N_A2_SC and n2b_zero:
                nc.scalar.activation(xs[i], prexs[i], AF.Identity,
                                     scale=rs2c, bias=nm2c)
            else:
                nc.vector.tensor_scalar(
                    xs[i], prexs[i], scalar1=prev["mv2"][:, i:i + 1],
                    scalar2=rs2c, op0=ALU.subtract, op1=ALU.mult,
                )
                if not n2b_zero:
                    nc.vector.tensor_scalar_add(
                        xs[i], xs[i], t_n2b[:, prev["l"]:prev["l"] + 1])

        for l in range(L):
            wq_l = t_wq[:, l * E:(l + 1) * E]
            wkv_l = t_wkv[:, l * 2 * E:(l + 1) * 2 * E]

            # ---- phase A: qkv + exp/tanh (+ prev layer inorm2 apply) ----
            for i in range(BI):
                if l > 0:
                    emit_apply2(i)
                else:
                    emit_phase0(i)
                if not alphas_equal:
                    load_adT(i, s_enc[l])
                ps_kv = pp.tile([128, 1024], F32, tag="kv")
                for t in range(NT):
                    xt = xs[i][:, t * 128:(t + 1) * 128]
                    nc.tensor.matmul(ps_kv[:, t * 256:(t + 1) * 256],
                                     lhsT=xt, rhs=wkv_l, start=True, stop=True)
                kv3 = ps_kv.rearrange("p (t c) -> p t c", c=256)
                ps_q = pq.tile([128, 512], F32, tag="q")
                nc.tensor.matmul(ps_q, lhsT=wq_l, rhs=xs[i],
                                 start=True, stop=True)
                eks[i] = pers.tile([128, 512], BF, tag=f"ek{i}", name=f"ek{i}")
                ek3 = eks[i].rearrange("p (t c) -> p t c", c=E)
                nc.scalar.activation(ek3, kv3[:, :, 0:E], AF.Exp)
                ths[i] = pers.tile([128, 512], BF, tag=f"th{i}", name=f"th{i}")
                nc.scalar.activation(ths[i], ps_q, AF.Tanh, scale=0.5)
                ekvs[i] = pers.tile([128, 512], BF, tag=f"ekv{i}", name=f"ekv{i}")
                # ekv = (0.5*v) * ek   (0.5 is the sigmoid-from-tanh factor)
                nc.vector.scalar_tensor_tensor(
                    ekvs[i].rearrange("p (t c) -> p t c", c=E),
                    in0=kv3[:, :, E:2 * E], scalar=0.5, in1=ek3,
                    op0=ALU.mult, op1=ALU.mult,
                )

            if l == 0:
                nc.sync.dma_start(t_w1, d_w1[:])
                nc.sync.dma_start(t_w2, d_w2[:])

            # ---- phases B & C interleaved per instance (convoy killer) ----
            mvall1 = pers.tile([128, 16], F32, tag="mvall1", name=f"mvall1_{l}")
            mv31 = mvall1.rearrange("p (two i) -> p two i", i=8)
            mvall2 = pers.tile([128, 16], F32, tag="mvall2", name=f"mvall2_{l}")
            mv32 = mvall2.rearrange("p (two i) -> p two i", i=8)
            rs1 = {}

            def emit_B(i):
                ps_den = pn.tile([128, 512], F32, tag="nd")
                ps_num = pn.tile([128, 512], F32, tag="nd")
                for t in range(NT):
                    nc.tensor.matmul(
                        ps_den, lhsT=eks[i][:, t * 128:(t + 1) * 128],
                        rhs=adT[i][:, t * 512:(t + 1) * 512],
                        start=(t == 0), stop=(t == NT - 1),
                    )
                for t in range(NT):
                    nc.tensor.matmul(
                        ps_num, lhsT=ekvs[i][:, t * 128:(t + 1) * 128],
                        rhs=adT[i][:, t * 512:(t + 1) * 512],
                        start=(t == 0), stop=(t == NT - 1),
                    )
                rden = wk3.tile([128, 512], F32, tag="rden")
                nc.vector.reciprocal_approx_fast(rden, ps_den)
                w_ = wk6.tile([128, 512], BF, tag="w")
                nc.vector.tensor_tensor(w_, ps_num, rden, op=ALU.mult)
                # u = (th + 1) * w  (0.5 folded into ekv)
                u_ = wk6.tile([128, 512], BF, tag="u")
                nc.vector.scalar_tensor_tensor(
                    u_, in0=ths[i], scalar=1.0, in1=w_,
                    op0=ALU.add, op1=ALU.mult,
                )
                prehs[i] = pers.tile([128, 512], F32, tag=f"pre{i}", name=f"preh{i}")
                nc.gpsimd.tensor_tensor(prehs[i], u_, xs[i], op=ALU.add)
                stat = wk2.tile([128, 6], F32, tag="stat")
                nc.vector.bn_stats(stat, prehs[i])
                nc.vector.bn_aggr(mv31[:, :, i:i + 1], stat)
                if i == 3 or i == 7:
                    lo = i - 3
                    rr = batched_rsqrt(mvall1, f"n1{'ab'[lo > 0]}_{l}", lo, i + 1)
                    if not n1w_ones:
                        rw = wk2.tile([128, 4], F32, tag=f"rq_w{i}",
                                      name=f"rs1w{i}_{l}")
                        nc.vector.tensor_scalar_mul(rw, rr, t_n1w[:, l:l + 1])
                        rr = rw
                    rs1["a" if i == 3 else "b"] = rr
                    if N_H_SC > 0:
                        nm1 = wk2.tile([128, 4], F32, tag=f"nm1{'ab'[lo > 0]}",
                                       name=f"nm1{'ab'[lo > 0]}_{l}")
                        nc.vector.scalar_tensor_tensor(
                            nm1, in0=mvall1[:, lo:lo + 4], scalar=-1.0,
                            in1=rr, op0=ALU.mult, op1=ALU.mult,
                        )
                        rs1["na" if i == 3 else "nb"] = nm1

            def emit_C(i):
                nonlocal rs2a, rs2b, nm2a, nm2b
                rs1c = (rs1["a"][:, i:i + 1] if i < 4
                        else rs1["b"][:, i - 4:i - 3])
                hs[i] = pers.tile([E, N], BF, tag=f"h{i}", name=f"h{i}")
                if i < N_H_SC and n1b_zero:
                    nm1c = (rs1["na"][:, i:i + 1] if i < 4
                            else rs1["nb"][:, i - 4:i - 3])
                    nc.scalar.activation(hs[i], prehs[i], AF.Identity,
                                         scale=rs1c, bias=nm1c)
                else:
                    nc.vector.tensor_scalar(
                        hs[i], prehs[i], scalar1=mvall1[:, i:i + 1],
                        scalar2=rs1c,
                        op0=ALU.subtract, op1=ALU.mult,
                    )
                    if not n1b_zero:
                        nc.vector.tensor_scalar_add(hs[i], hs[i],
                                                    t_n1b[:, l:l + 1])
                ps_f2 = pq.tile([128, 512], F32, tag="q")
                for ft in range(FT):
                    ps_f = pn.tile([128, 512], F32, tag="nd")
                    nc.tensor.matmul(
                        ps_f,
                        lhsT=t_w1[:, l * F + ft * 128:l * F + (ft + 1) * 128],
                        rhs=hs[i], start=True, stop=True,
                    )
                    g = wk4.tile([128, 512], BF, tag="g")
                    b1c = None if b1_zero else t_b1[:, l * FT + ft:l * FT + ft + 1]
                    if RELU_ENG[ft] == "s":
                        if b1c is None:
                            nc.scalar.activation(g, ps_f, AF.Relu)
                        else:
                            nc.scalar.activation(g, ps_f, AF.Relu, bias=b1c)
                    else:
                        if b1c is None:
                            nc.vector.tensor_scalar_max(g, ps_f, 0.0)
                        else:
                            nc.vector.tensor_scalar(
                                g, ps_f, scalar1=b1c, scalar2=0.0,
                                op0=ALU.add, op1=ALU.max,
                            )
                    nc.tensor.matmul(
                        ps_f2,
                        lhsT=t_w2[:, (l * FT + ft) * E:(l * FT + ft + 1) * E],
                        rhs=g, start=(ft == 0), stop=(ft == FT - 1),
                    )
                prexs[i] = pers.tile([128, 512], F32, tag=f"pre{i}", name=f"prex{i}")
                if b2_zero:
                    nc.vector.scalar_tensor_tensor(
                        prexs[i], in0=ps_f2, scalar=1.0, in1=hs[i],
                        op0=ALU.bypass, op1=ALU.add,
                    )
                else:
                    nc.vector.scalar_tensor_tensor(
                        prexs[i], in0=ps_f2, scalar=t_b2[:, l:l + 1],
                        in1=hs[i], op0=ALU.add, op1=ALU.add,
                    )
                stat = wk2.tile([128, 6], F32, tag="stat")
                nc.vector.bn_stats(stat, prexs[i])
                nc.vector.bn_aggr(mv32[:, :, i:i + 1], stat)
                if i == 3 or i == 7:
                    lo = i - 3
                    rr = batched_rsqrt(mvall2, f"n2{'ab'[lo > 0]}_{l}",
                                       lo, i + 1)
                    if not n2w_ones:
                        rw = wk2.tile([128, 4], F32, tag=f"rq_w2{i}",
                                      name=f"rs2w{i}_{l}")
                        nc.vector.tensor_scalar_mul(rw, rr, t_n2w[:, l:l + 1])
                        rr = rw
                    nmw = wk2.tile([128, 4], F32, tag=f"nm2{'ab'[lo > 0]}",
                                   name=f"nm2{'ab'[lo > 0]}_{l}")
                    nc.vector.scalar_tensor_tensor(
                        nmw, in0=mvall2[:, lo:lo + 4], scalar=-1.0, in1=rr,
                        op0=ALU.mult, op1=ALU.mult,
                    )
                    if i == 3:
                        rs2a, nm2a = rr, nmw
                    else:
                        rs2b, nm2b = rr, nmw

            rs2a = rs2b = nm2a = nm2b = None
            for i in range(BI):
                emit_B(i)
            for i in range(BI):
                emit_C(i)
            prev["mv2"], prev["rs2a"], prev["rs2b"], prev["l"] = \
                mvall2, rs2a, rs2b, l
            prev["nm2a"], prev["nm2b"] = nm2a, nm2b

        # ---------------- decoder ----------------
        # dA for all instances (deep pipeline), then dB+dC per instance.
        thd, aftd = {}, {}
        dec_q_rank1 = n2w_ones and n2b_zero  # enc mean over nodes == 0
        for i in range(BI):
            emit_apply2(i)
            adT[i] = pers.tile([128, NT * P], BF, tag=f"adT{i}", name=f"adTd{i}")
            for t in range(NT):
                nc.sync.dma_start(adT[i][:, t * P:(t + 1) * P],
                                  d_cdT[i, t * 128:(t + 1) * 128, :])
            ps_kv = pp.tile([128, 1024], F32, tag="kv")
            for t in range(NT):
                xt = xs[i][:, t * 128:(t + 1) * 128]
                nc.tensor.matmul(ps_kv[:, t * 256:(t + 1) * 256],
                                 lhsT=xt, rhs=t_dkv, start=True, stop=True)
            kv3 = ps_kv.rearrange("p (t c) -> p t c", c=256)
            eks[i] = pers.tile([128, 512], BF, tag=f"ek{i}", name=f"ek{i}")
            ek3 = eks[i].rearrange("p (t c) -> p t c", c=E)
            nc.scalar.activation(ek3, kv3[:, :, 0:E], AF.Exp)
            ekvs[i] = pers.tile([128, 512], BF, tag=f"ekv{i}", name=f"ekv{i}")
            # fold sigmoid 0.5 and 1/sqrt(E) of the score into ekv
            nc.vector.scalar_tensor_tensor(
                ekvs[i].rearrange("p (t c) -> p t c", c=E),
                in0=kv3[:, :, E:2 * E], scalar=0.5 / SQRT_E, in1=ek3,
                op0=ALU.mult, op1=ALU.mult,
            )
            cap_t = wk2.tile([1, P], F32R, tag="cap")
            nc.sync.dma_start(cap_t, d_cap[i:i + 1, :])
            ps_qd = pq.tile([128, 512], F32, tag="q")
            nc.tensor.matmul(ps_qd, lhsT=t_dwql, rhs=cap_t,
                             start=True, stop=True)
            thd[i] = pers.tile([128, 512], BF, tag=f"th{i}", name=f"thd{i}")
            if dec_q_rank1:
                nc.scalar.activation(thd[i], ps_qd, AF.Tanh, scale=0.5)
            else:
                gsum = wk2.tile([128, 1], BF, tag="gsum")
                nc.vector.tensor_reduce(gsum, xs[i], axis=AX.X, op=ALU.add)
                ps_g = pn.tile([128, 512], F32, tag="nd")
                nc.tensor.matmul(ps_g[:, 0:1], lhsT=t_dwqm, rhs=gsum,
                                 start=True, stop=True)
                gqh = wk2.tile([128, 1], F32, tag="gqh")
                nc.vector.tensor_scalar_mul(gqh, ps_g[:, 0:1], 0.5 / N)
                nc.scalar.activation(thd[i], ps_qd, AF.Tanh, scale=0.5,
                                     bias=gqh[:, 0:1])

        for i in range(BI):
            # dB: dec num/den + aft (den first: rden is the DVE gate)
            ps_den = pn.tile([128, 512], F32, tag="nd")
            ps_num = pn.tile([128, 512], F32, tag="nd")
            for t in range(NT):
                nc.tensor.matmul(
                    ps_den, lhsT=eks[i][:, t * 128:(t + 1) * 128],
                    rhs=adT[i][:, t * 512:(t + 1) * 512],
                    start=(t == 0), stop=(t == NT - 1),
                )
            for t in range(NT):
                nc.tensor.matmul(
                    ps_num, lhsT=ekvs[i][:, t * 128:(t + 1) * 128],
                    rhs=adT[i][:, t * 512:(t + 1) * 512],
                    start=(t == 0), stop=(t == NT - 1),
                )
            rden = wk3.tile([128, 512], F32, tag="rden")
            nc.vector.reciprocal_approx_fast(rden, ps_den)
            w_ = wk2.tile([128, 512], BF, tag="w")
            nc.vector.tensor_tensor(w_, ps_num, rden, op=ALU.mult)
            aftd[i] = pers.tile([128, 512], BF, tag=f"pre{i}", name=f"aftd{i}")
            nc.vector.scalar_tensor_tensor(
                aftd[i], in0=thd[i], scalar=1.0, in1=w_,
                op0=ALU.add, op1=ALU.mult,
            )

            # dC: score + softmax (cur_dist pre-scaled by s_probs on host;
            # added into the score PSUM via an identity-matmul accumulate)
            if use_ninf:
                nstg = wk2.tile([128, PT * N], F32, tag="nstage")
                nc.sync.dma_start(
                    nstg.rearrange("p (t d) -> p t d", d=N),
                    d_ninf[i].rearrange("(t p) d -> p t d", p=128),
                )
            for pt in range(PT):
                cdc = wk4.tile([128, N], BF, tag="cdc")
                nc.sync.dma_start(cdc, d_cd[i, pt * 128:(pt + 1) * 128, :])
                ps_s = pn.tile([128, 512], F32, tag="nd")
                nc.tensor.matmul(ps_s, lhsT=aftd[i][:, pt * 128:(pt + 1) * 128],
                                 rhs=xs[i], start=True, stop=False)
                nc.tensor.matmul(ps_s, lhsT=t_ident, rhs=cdc,
                                 start=False, stop=True)
                th2 = wk6.tile([128, 512], F32, tag="th2")
                nc.scalar.activation(th2, ps_s, AF.Tanh)
                es = wk4.tile([128, 512], F32, tag="es")
                ssum = wk2.tile([128, 1], F32, tag="ssum")
                if use_ninf:
                    thm = wk2.tile([128, 512], F32, tag="thm")
                    nc.vector.scalar_tensor_tensor(
                        thm, in0=th2, scalar=CLIP,
                        in1=nstg[:, pt * N:(pt + 1) * N],
                        op0=ALU.mult, op1=ALU.add,
                    )
                    nc.scalar.activation(es, thm, AF.Exp, accum_out=ssum)
                else:
                    nc.scalar.activation(es, th2, AF.Exp, scale=CLIP,
                                         accum_out=ssum)
                o = wk4.tile([128, 512], F32, tag="o")
                nc.gpsimd.normalize_recip(o, es, ssum[:, 0:1])
                nc.sync.dma_start(d_out[i, pt * 128:(pt + 1) * 128, :], o)

    return nc


def _prep_maps(inputs):
    f32 = lambda a: np.ascontiguousarray(np.asarray(a, dtype=np.float32))
    bf = lambda a: np.ascontiguousarray(
        np.asarray(a, dtype=np.float32).astype(ml_dtypes.bfloat16))
    data = f32(inputs["data"])
    dist = f32(inputs["dist"])
    cur_dist = f32(inputs["cur_dist"])
    capacity = f32(inputs["capacity"])
    ninf = f32(inputs["ninf_mask"])
    meta = {
        "ls": float(np.asarray(inputs["log_scale"]).reshape(-1)[0]),
        "aft_alpha": [float(v) for v in np.asarray(inputs["aft_alpha"])],
        "dec_aft_alpha": float(np.asarray(inputs["dec_aft_alpha"]).reshape(-1)[0]),
        "probs_alpha": float(np.asarray(inputs["probs_alpha"]).reshape(-1)[0]),
        "use_ninf": bool(np.any(ninf)),
        "n1w_ones": bool(np.all(inputs["n1_w"] == 1.0)),
        "n1b_zero": not bool(np.any(inputs["n1_b"])),
        "n2w_ones": bool(np.all(inputs["n2_w"] == 1.0)),
        "n2b_zero": not bool(np.any(inputs["n2_b"])),
        "b1_zero": not bool(np.any(inputs["ff_b1"])),
        "b2_zero": not bool(np.any(inputs["ff_b2"])),
        "embb_zero": not bool(np.any(inputs["emb_b"])),
    }
    wq = f32(inputs["Wq"]).transpose(1, 0, 2).reshape(E, L * E)
    wkv = np.concatenate([f32(inputs["Wk"]), f32(inputs["Wv"])], axis=2)
    wkv = wkv.transpose(1, 0, 2).reshape(E, L * 2 * E)
    dkv = np.concatenate([f32(inputs["dWk"]), f32(inputs["dWv"])], axis=1)
    w1 = f32(inputs["ff_W1"]).transpose(1, 0, 2).reshape(E, L * F)
    w2 = (f32(inputs["ff_W2"]).reshape(L, FT, 128, E)
          .transpose(2, 0, 1, 3).reshape(128, L * FT * E))
    dwq = f32(inputs["dWq"])
    shared = {
        "emb_w": f32(inputs["emb_W"]),
        "wq_p": bf(wq), "wkv_p": bf(wkv),
        "w1_p": bf(w1), "w2_p": bf(w2),
        "dwq_main": bf(dwq[:E]),
        "ident": np.eye(128, dtype=np.float32).astype(ml_dtypes.bfloat16),
        "dwq_last": f32(dwq[E:E + 1]),
        "dkv": bf(dkv),
    }
    if not meta["embb_zero"]:
        shared["emb_b"] = f32(inputs["emb_b"]).reshape(E, 1)
    if not meta["n1w_ones"]:
        shared["n1wp"] = np.ascontiguousarray(f32(inputs["n1_w"]).T)
    if not meta["n1b_zero"]:
        shared["n1bp"] = np.ascontiguousarray(f32(inputs["n1_b"]).T)
    if not meta["n2w_ones"]:
        shared["n2wp"] = np.ascontiguousarray(f32(inputs["n2_w"]).T)
    if not meta["n2b_zero"]:
        shared["n2bp"] = np.ascontiguousarray(f32(inputs["n2_b"]).T)
    if not meta["b1_zero"]:
        shared["b1p"] = np.ascontiguousarray(
            f32(inputs["ff_b1"]).reshape(L, FT, 128).transpose(2, 0, 1)
            .reshape(128, L * FT))
    if not meta["b2_zero"]:
        shared["b2p"] = np.ascontiguousarray(f32(inputs["ff_b2"]).T)
    in_maps = []
    for c in range(NCORES):
        s = slice(c * BI, (c + 1) * BI)
        m = dict(shared)
        m["data_t"] = np.ascontiguousarray(data[s].transpose(0, 2, 1))
        s_enc0 = meta["ls"] * meta["aft_alpha"][0]
        s_dec = meta["ls"] * meta["dec_aft_alpha"]
        m["dist_t"] = bf(np.exp(s_enc0 * dist[s].transpose(0, 2, 1)))
        m["cur_dist"] = bf(meta["ls"] * meta["probs_alpha"] * cur_dist[s])
        m["cur_dist_t"] = bf(np.exp(
            s_dec * cur_dist[s].transpose(0, 2, 1)
            + ninf[s].transpose(0, 2, 1)))
        m["capacity"] = capacity[s]
        if meta["use_ninf"]:
            m["ninf"] = ninf[s]
        in_maps.append(m)
    return in_maps, meta


def kernel(_trace=False, **inputs):
    in_maps, meta = _prep_maps(inputs)
    nc = bacc.Bacc(None)
    _build(nc, meta)
    nc.finalize()
    res = run_bass_kernel_spmd(nc, in_maps, list(range(NCORES)), trace=_trace)
    out = np.concatenate([res.results[c]["out"] for c in range(NCORES)], axis=0)
    if _trace:
        return out.astype(np.float32), res
    return out.astype(np.float32)



# revision 20
# speedup vs baseline: 1.0544x; 1.0544x over previous
"""Trainium2 Bass kernel for nn_KPModel (AFT encoder + AFT decoder + softmax).

v2 strategy (vs v1's per-instance serial program):
- 8 instances per core processed in LAYER LOCKSTEP: each phase (qkv,
  aft, norm+ff, ...) runs across all 8 instances before moving on.  This
  batches scalar-engine activations by table set (exp/tanh/relu vs sqrt)
  so ACT_TABLE_LOADs drop from ~21/instance to ~2/layer, and gives every
  engine 8 independent dependency chains to pipeline.
- bf16 operands for all matmuls and most elementwise ops (PSUM stays
  f32): 2-4x DVE throughput, FWL weight loads, half SBUF footprint.
- sigmoid(x) -> 0.5*(1+tanh(x/2)): tanh lives in the same activation
  table set as exp/relu, eliminating the sigmoid table set entirely.
- PSUM discipline: 4 tags x 2 bufs x [128,512]f32 = exactly 8 banks.
- decoder query via rank-1 trick: q = dWq_m^T gmean (+) dwq_last (x) cap,
  computed as a 1-col matmul + a K=1 rank-1 matmul + activation bias.
"""

import numpy as np
import ml_dtypes

try:
    import concourse.bass as bass  # noqa: F401
except ImportError:  # pragma: no cover
    import sys

    sys.path.insert(0, "/opt/trn_rl_repo")
    import concourse.bass as bass

from contextlib import ExitStack

from concourse import bacc
import concourse.mybir as mybir
from concourse.bass_utils import run_bass_kernel_spmd
from concourse.tile import TileContext

F32 = mybir.dt.float32
F32R = mybir.dt.float32r
BF = mybir.dt.bfloat16
AF = mybir.ActivationFunctionType
ALU = mybir.AluOpType
AX = mybir.AxisListType
I32 = mybir.dt.int32
RSQRT_MAGIC = 0x5F3759DF

B, P, N, E, F, L = 64, 512, 512, 128, 512, 6
SQRT_E = 11.313708498984761
CLIP = 10.0
EPS = 1e-5
NCORES = 8
BI = B // NCORES
NT = N // 128
PT = P // 128
FT = F // 128

# which engine runs the relu for each of the 4 FF tiles ("s"=scalar, "v"=DVE)
RELU_ENG = ("s", "s", "s", "s")
# how many instances run the inorm applies on the scalar engine (rest: DVE)
N_A2_SC = 8
N_H_SC = 8


def _build(nc, meta):
    ls = meta["ls"]
    s_enc = [ls * a for a in meta["aft_alpha"]]
    alphas_equal = all(abs(s - s_enc[0]) < 1e-12 for s in s_enc)
    assert alphas_equal, "host pre-exp requires equal aft alphas"
    s_dec = ls * meta["dec_aft_alpha"]
    s_probs = ls * meta["probs_alpha"]
    use_ninf = meta["use_ninf"]
    n1w_ones, n1b_zero = meta["n1w_ones"], meta["n1b_zero"]
    n2w_ones, n2b_zero = meta["n2w_ones"], meta["n2b_zero"]
    b1_zero, b2_zero = meta["b1_zero"], meta["b2_zero"]
    embb_zero = meta["embb_zero"]

    # ---------------- DRAM I/O ----------------
    d_data = nc.dram_tensor("data_t", [BI, 2, N], F32R, kind="ExternalInput")
    d_distT = nc.dram_tensor("dist_t", [BI, N, N], BF, kind="ExternalInput")
    d_cd = nc.dram_tensor("cur_dist", [BI, P, N], BF, kind="ExternalInput")
    d_cdT = nc.dram_tensor("cur_dist_t", [BI, N, P], BF, kind="ExternalInput")
    d_cap = nc.dram_tensor("capacity", [BI, P], F32R, kind="ExternalInput")
    if use_ninf:
        d_ninf = nc.dram_tensor("ninf", [BI, P, N], F32, kind="ExternalInput")
    d_embw = nc.dram_tensor("emb_w", [2, E], F32R, kind="ExternalInput")
    d_embb = None
    if not embb_zero:
        d_embb = nc.dram_tensor("emb_b", [E, 1], F32, kind="ExternalInput")
    d_wq = nc.dram_tensor("wq_p", [E, L * E], BF, kind="ExternalInput")
    d_wkv = nc.dram_tensor("wkv_p", [E, L * 2 * E], BF, kind="ExternalInput")
    d_w1 = nc.dram_tensor("w1_p", [E, L * F], BF, kind="ExternalInput")
    d_w2 = nc.dram_tensor("w2_p", [128, L * FT * E], BF, kind="ExternalInput")
    d_n1w = d_n1b = d_n2w = d_n2b = d_b1 = d_b2 = None
    if not n1w_ones:
        d_n1w = nc.dram_tensor("n1wp", [E, L], F32, kind="ExternalInput")
    if not n1b_zero:
        d_n1b = nc.dram_tensor("n1bp", [E, L], F32, kind="ExternalInput")
    if not n2w_ones:
        d_n2w = nc.dram_tensor("n2wp", [E, L], F32, kind="ExternalInput")
    if not n2b_zero:
        d_n2b = nc.dram_tensor("n2bp", [E, L], F32, kind="ExternalInput")
    if not b1_zero:
        d_b1 = nc.dram_tensor("b1p", [128, L * FT], F32, kind="ExternalInput")
    if not b2_zero:
        d_b2 = nc.dram_tensor("b2p", [E, L], F32, kind="ExternalInput")
    d_dwqm = nc.dram_tensor("dwq_main", [E, E], BF, kind="ExternalInput")
    d_ident = nc.dram_tensor("ident", [128, 128], BF, kind="ExternalInput")
    d_dwql = nc.dram_tensor("dwq_last", [1, E], F32R, kind="ExternalInput")
    d_dkv = nc.dram_tensor("dkv", [E, 2 * E], BF, kind="ExternalInput")
    d_out = nc.dram_tensor("out", [BI, P, N], F32, kind="ExternalOutput")

    with TileContext(nc) as tc, ExitStack() as es, \
            nc.allow_low_precision("bf16 activations by design; fp32 psum"):
        cw = es.enter_context(tc.tile_pool(name="cw", bufs=1))
        pers = es.enter_context(tc.tile_pool(name="pers", bufs=1))
        wk2 = es.enter_context(tc.tile_pool(name="wk2", bufs=2))
        wk3 = es.enter_context(tc.tile_pool(name="wk3", bufs=3))
        wk4 = es.enter_context(tc.tile_pool(name="wk4", bufs=4))
        wk6 = es.enter_context(tc.tile_pool(name="wk6", bufs=6))
        pp = es.enter_context(tc.tile_pool(name="pp", bufs=2, space="PSUM"))
        pq = es.enter_context(tc.tile_pool(name="pq", bufs=1, space="PSUM"))
        pn = es.enter_context(tc.tile_pool(name="pn", bufs=3, space="PSUM"))

        # ---------------- weights ----------------
        t_embw = cw.tile([2, E], F32R)
        nc.sync.dma_start(t_embw, d_embw[:])
        t_embb = None
        if not embb_zero:
            t_embb = cw.tile([E, 1], F32)
            nc.sync.dma_start(t_embb, d_embb[:])
        t_wq = cw.tile([E, L * E], BF)
        nc.sync.dma_start(t_wq, d_wq[:])
        t_wkv = cw.tile([E, L * 2 * E], BF)
        nc.sync.dma_start(t_wkv, d_wkv[:])
        t_w1 = cw.tile([E, L * F], BF)
        t_w2 = cw.tile([128, L * FT * E], BF)

        def _maybe(dram, shape):
            if dram is None:
                return None
            t = cw.tile(shape, F32)
            nc.sync.dma_start(t, dram[:])
            return t

        t_n1w = _maybe(d_n1w, [E, L])
        t_n1b = _maybe(d_n1b, [E, L])
        t_n2w = _maybe(d_n2w, [E, L])
        t_n2b = _maybe(d_n2b, [E, L])
        t_b1 = _maybe(d_b1, [128, L * FT])
        t_b2 = _maybe(d_b2, [E, L])
        t_dwqm = cw.tile([E, E], BF)
        nc.sync.dma_start(t_dwqm, d_dwqm[:])
        t_ident = cw.tile([128, 128], BF)
        nc.sync.dma_start(t_ident, d_ident[:])
        t_dwql = cw.tile([1, E], F32R)
        nc.sync.dma_start(t_dwql, d_dwql[:])
        t_dkv = cw.tile([E, 2 * E], BF)
        nc.sync.dma_start(t_dkv, d_dkv[:])
        t_eps = cw.tile([128, 1], F32)
        nc.vector.memset(t_eps, EPS)

        # per-instance persistent tiles (tag-addressed, bufs=1)
        adT = {}
        xs = {}

        def load_adT(i, scale):
            """DMA the host-preexponentiated adT slab for instance i."""
            adT[i] = pers.tile([128, NT * N], BF, tag=f"adT{i}", name=f"adT{i}")
            for t in range(NT):
                nc.sync.dma_start(adT[i][:, t * N:(t + 1) * N],
                                  d_distT[i, t * 128:(t + 1) * 128, :])

        def emit_phase0(i):
            """Embedding then adT for instance i (fused into layer 0 phase A).

            dat/emb first so the PE's first matmul only waits on two tiny
            DMAs, not the 1MB dist slab."""
            dat = wk2.tile([2, N], F32R, tag="dat")
            nc.sync.dma_start(dat, d_data[i])
            ps_e = pq.tile([128, 512], F32, tag="q")
            nc.tensor.matmul(ps_e, lhsT=t_embw, rhs=dat, start=True, stop=True)
            xs[i] = pers.tile([E, N], BF, tag=f"x{i}", name=f"x{i}")
            if embb_zero:
                nc.scalar.activation(xs[i], ps_e, AF.Copy)
            else:
                nc.scalar.activation(xs[i], ps_e, AF.Identity,
                                     bias=t_embb[:, 0:1])
            if alphas_equal:
                load_adT(i, s_enc[0])

        # ---------------- encoder layers ----------------
        eks, ekvs, ths, prehs, prexs = {}, {}, {}, {}, {}
        hs = {}

        def batched_rsqrt(mvall, tagp, lo, hi):
            """rs[:, i-lo] = 1/sqrt(var_i + eps) for instances lo..hi-1.

            Quake bit-trick seed + 2 Newton iterations, all on DVE int/f32
            ALU ops, so the scalar engine never needs the sqrt table set.
            """
            nb = hi - lo
            ve = wk2.tile([128, nb], F32, tag=f"rq_ve{lo}", name=f"ve_{tagp}")
            nc.vector.tensor_scalar(ve, mvall[:, 8 + lo:8 + hi], scalar1=EPS,
                                    scalar2=None, op0=ALU.add)
            tb = wk2.tile([128, nb], I32, tag=f"rq_tb{lo}", name=f"tb_{tagp}")
            nc.vector.tensor_scalar(tb, ve.bitcast(I32), scalar1=1,
                                    scalar2=None, op0=ALU.arith_shift_right)
            sd = wk2.tile([128, nb], I32, tag=f"rq_sd{lo}", name=f"sd_{tagp}")
            nc.vector.tensor_scalar(sd, tb, scalar1=-1, scalar2=RSQRT_MAGIC,
                                    op0=ALU.mult, op1=ALU.add)
            y = sd.bitcast(F32)
            for it in range(1):
                a = wk2.tile([128, nb], I32 if False else F32,
                             tag=f"rq_a{it}{lo}", name=f"a{it}_{tagp}")
                nc.vector.tensor_tensor(a, y, y, op=ALU.mult)
                b = wk2.tile([128, nb], F32, tag=f"rq_b{it}{lo}",
                             name=f"b{it}_{tagp}")
                nc.vector.tensor_tensor(b, a, ve, op=ALU.mult)
                c = wk2.tile([128, nb], F32, tag=f"rq_c{it}{lo}",
                             name=f"c{it}_{tagp}")
                nc.vector.tensor_scalar(c, b, scalar1=-0.5, scalar2=1.5,
                                        op0=ALU.mult, op1=ALU.add)
                y2 = wk2.tile([128, nb], F32, tag=f"rq_y{it}{lo}",
                              name=f"y{it}_{tagp}")
                nc.vector.tensor_tensor(y2, y, c, op=ALU.mult)
                y = y2
            return y

        prev = {}

        def emit_apply2(i):
            if i < 4:
                rs2c = prev["rs2a"][:, i:i + 1]
                nm2c = prev["nm2a"][:, i:i + 1]
            else:
                rs2c = prev["rs2b"][:, i - 4:i - 3]
                nm2c = prev["nm2b"][:, i - 4:i - 3]
            xs[i] = pers.tile([E, N], BF, tag=f"x{i}", name=f"x{i}")
            if i < N_A2_SC and n2b_zero:
                nc.scalar.activation(xs[i], prexs[i], AF.Identity,
                                     scale=rs2c, bias=nm2c)
            else:
                nc.vector.tensor_scalar(
                    xs[i], prexs[i], scalar1=prev["mv2"][:, i:i + 1],
                    scalar2=rs2c, op0=ALU.subtract, op1=ALU.mult,
                )
                if not n2b_zero:
                    nc.vector.tensor_scalar_add(
                        xs[i], xs[i], t_n2b[:, prev["l"]:prev["l"] + 1])

        for l in range(L):
            wq_l = t_wq[:, l * E:(l + 1) * E]
            wkv_l = t_wkv[:, l * 2 * E:(l + 1) * 2 * E]

            # ---- phase A: qkv + exp/tanh (+ prev layer inorm2 apply) ----
            for i in range(BI):
                if l > 0:
                    emit_apply2(i)
                else:
                    emit_phase0(i)
                if not alphas_equal:
                    load_adT(i, s_enc[l])
                ps_kv = pp.tile([128, 1024], F32, tag="kv")
                for t in range(NT):
                    xt = xs[i][:, t * 128:(t + 1) * 128]
                    nc.tensor.matmul(ps_kv[:, t * 256:(t + 1) * 256],
                                     lhsT=xt, rhs=wkv_l, start=True, stop=True)
                kv3 = ps_kv.rearrange("p (t c) -> p t c", c=256)
                ps_q = pq.tile([128, 512], F32, tag="q")
                nc.tensor.matmul(ps_q, lhsT=wq_l, rhs=xs[i],
                                 start=True, stop=True)
                eks[i] = pers.tile([128, 512], BF, tag=f"ek{i}", name=f"ek{i}")
                ek3 = eks[i].rearrange("p (t c) -> p t c", c=E)
                nc.scalar.activation(ek3, kv3[:, :, 0:E], AF.Exp)
                ths[i] = pers.tile([128, 512], BF, tag=f"th{i}", name=f"th{i}")
                nc.scalar.activation(ths[i], ps_q, AF.Tanh, scale=0.5)
                ekvs[i] = pers.tile([128, 512], BF, tag=f"ekv{i}", name=f"ekv{i}")
                # ekv = (0.5*v) * ek   (0.5 is the sigmoid-from-tanh factor)
                nc.vector.scalar_tensor_tensor(
                    ekvs[i].rearrange("p (t c) -> p t c", c=E),
                    in0=kv3[:, :, E:2 * E], scalar=0.5, in1=ek3,
                    op0=ALU.mult, op1=ALU.mult,
                )

            if l == 0:
                nc.sync.dma_start(t_w1, d_w1[:])
                nc.sync.dma_start(t_w2, d_w2[:])

            # ---- phases B & C interleaved per instance (convoy killer) ----
            mvall1 = pers.tile([128, 16], F32, tag="mvall1", name=f"mvall1_{l}")
            mv31 = mvall1.rearrange("p (two i) -> p two i", i=8)
            mvall2 = pers.tile([128, 16], F32, tag="mvall2", name=f"mvall2_{l}")
            mv32 = mvall2.rearrange("p (two i) -> p two i", i=8)
            rs1 = {}

            def emit_B(i):
                ps_den = pn.tile([128, 512], F32, tag="nd")
                ps_num = pn.tile([128, 512], F32, tag="nd")
                for t in range(NT):
                    nc.tensor.matmul(
                        ps_den, lhsT=eks[i][:, t * 128:(t + 1) * 128],
                        rhs=adT[i][:, t * 512:(t + 1) * 512],
                        start=(t == 0), stop=(t == NT - 1),
                    )
                for t in range(NT):
                    nc.tensor.matmul(
                        ps_num, lhsT=ekvs[i][:, t * 128:(t + 1) * 128],
                        rhs=adT[i][:, t * 512:(t + 1) * 512],
                        start=(t == 0), stop=(t == NT - 1),
                    )
                rden = wk3.tile([128, 512], F32, tag="rden")
                nc.vector.reciprocal_approx_fast(rden, ps_den)
                w_ = wk6.tile([128, 512], BF, tag="w")
                nc.vector.tensor_tensor(w_, ps_num, rden, op=ALU.mult)
                # u = (th + 1) * w  (0.5 folded into ekv)
                u_ = wk6.tile([128, 512], BF, tag="u")
                nc.vector.scalar_tensor_tensor(
                    u_, in0=ths[i], scalar=1.0, in1=w_,
                    op0=ALU.add, op1=ALU.mult,
                )
                prehs[i] = pers.tile([128, 512], F32, tag=f"pre{i}", name=f"preh{i}")
                nc.gpsimd.tensor_tensor(prehs[i], u_, xs[i], op=ALU.add)
                stat = wk2.tile([128, 6], F32, tag="stat")
                nc.vector.bn_stats(stat, prehs[i])
                nc.vector.bn_aggr(mv31[:, :, i:i + 1], stat)
                if i == 3 or i == 7:
                    lo = i - 3
                    rr = batched_rsqrt(mvall1, f"n1{'ab'[lo > 0]}_{l}", lo, i + 1)
                    if not n1w_ones:
                        rw = wk2.tile([128, 4], F32, tag=f"rq_w{i}",
                                      name=f"rs1w{i}_{l}")
                        nc.vector.tensor_scalar_mul(rw, rr, t_n1w[:, l:l + 1])
                        rr = rw
                    rs1["a" if i == 3 else "b"] = rr
                    if N_H_SC > 0:
                        nm1 = wk2.tile([128, 4], F32, tag=f"nm1{'ab'[lo > 0]}",
                                       name=f"nm1{'ab'[lo > 0]}_{l}")
                        nc.vector.scalar_tensor_tensor(
                            nm1, in0=mvall1[:, lo:lo + 4], scalar=-1.0,
                            in1=rr, op0=ALU.mult, op1=ALU.mult,
                        )
                        rs1["na" if i == 3 else "nb"] = nm1

            def emit_C(i):
                nonlocal rs2a, rs2b, nm2a, nm2b
                rs1c = (rs1["a"][:, i:i + 1] if i < 4
                        else rs1["b"][:, i - 4:i - 3])
                hs[i] = pers.tile([E, N], BF, tag=f"h{i}", name=f"h{i}")
                if i < N_H_SC and n1b_zero:
                    nm1c = (rs1["na"][:, i:i + 1] if i < 4
                            else rs1["nb"][:, i - 4:i - 3])
                    nc.scalar.activation(hs[i], prehs[i], AF.Identity,
                                         scale=rs1c, bias=nm1c)
                else:
                    nc.vector.tensor_scalar(
                        hs[i], prehs[i], scalar1=mvall1[:, i:i + 1],
                        scalar2=rs1c,
                        op0=ALU.subtract, op1=ALU.mult,
                    )
                    if not n1b_zero:
                        nc.vector.tensor_scalar_add(hs[i], hs[i],
                                                    t_n1b[:, l:l + 1])
                ps_f2 = pq.tile([128, 512], F32, tag="q")
                for ft in range(FT):
                    ps_f = pn.tile([128, 512], F32, tag="nd")
                    nc.tensor.matmul(
                        ps_f,
                        lhsT=t_w1[:, l * F + ft * 128:l * F + (ft + 1) * 128],
                        rhs=hs[i], start=True, stop=True,
                    )
                    g = wk4.tile([128, 512], BF, tag="g")
                    b1c = None if b1_zero else t_b1[:, l * FT + ft:l * FT + ft + 1]
                    if RELU_ENG[ft] == "s":
                        if b1c is None:
                            nc.scalar.activation(g, ps_f, AF.Relu)
                        else:
                            nc.scalar.activation(g, ps_f, AF.Relu, bias=b1c)
                    else:
                        if b1c is None:
                            nc.vector.tensor_scalar_max(g, ps_f, 0.0)
                        else:
                            nc.vector.tensor_scalar(
                                g, ps_f, scalar1=b1c, scalar2=0.0,
                                op0=ALU.add, op1=ALU.max,
                            )
                    nc.tensor.matmul(
                        ps_f2,
                        lhsT=t_w2[:, (l * FT + ft) * E:(l * FT + ft + 1) * E],
                        rhs=g, start=(ft == 0), stop=(ft == FT - 1),
                    )
                prexs[i] = pers.tile([128, 512], F32, tag=f"pre{i}", name=f"prex{i}")
                if b2_zero:
                    nc.vector.scalar_tensor_tensor(
                        prexs[i], in0=ps_f2, scalar=1.0, in1=hs[i],
                        op0=ALU.bypass, op1=ALU.add,
                    )
                else:
                    nc.vector.scalar_tensor_tensor(
                        prexs[i], in0=ps_f2, scalar=t_b2[:, l:l + 1],
                        in1=hs[i], op0=ALU.add, op1=ALU.add,
                    )
                stat = wk2.tile([128, 6], F32, tag="stat")
                nc.vector.bn_stats(stat, prexs[i])
                nc.vector.bn_aggr(mv32[:, :, i:i + 1], stat)
                if i == 3 or i == 7:
                    lo = i - 3
                    rr = batched_rsqrt(mvall2, f"n2{'ab'[lo > 0]}_{l}",
                                       lo, i + 1)
                    if not n2w_ones:
                        rw = wk2.tile([128, 4], F32, tag=f"rq_w2{i}",
                                      name=f"rs2w{i}_{l}")
                        nc.vector.tensor_scalar_mul(rw, rr, t_n2w[:, l:l + 1])
                        rr = rw
                    nmw = wk2.tile([128, 4], F32, tag=f"nm2{'ab'[lo > 0]}",
                                   name=f"nm2{'ab'[lo > 0]}_{l}")
                    nc.vector.scalar_tensor_tensor(
                        nmw, in0=mvall2[:, lo:lo + 4], scalar=-1.0, in1=rr,
                        op0=ALU.mult, op1=ALU.mult,
                    )
                    if i == 3:
                        rs2a, nm2a = rr, nmw
                    else:
                        rs2b, nm2b = rr, nmw

            rs2a = rs2b = nm2a = nm2b = None
            for i in range(BI):
                emit_B(i)
            for i in range(BI):
                emit_C(i)
            prev["mv2"], prev["rs2a"], prev["rs2b"], prev["l"] = \
                mvall2, rs2a, rs2b, l
            prev["nm2a"], prev["nm2b"] = nm2a, nm2b

        # ---------------- decoder ----------------
        # dA for all instances (deep pipeline), then dB+dC per instance.
        thd, aftd = {}, {}
        dec_q_rank1 = n2w_ones and n2b_zero  # enc mean over nodes == 0
        for i in range(BI):
            emit_apply2(i)
            adT[i] = pers.tile([128, NT * P], BF, tag=f"adT{i}", name=f"adTd{i}")
            for t in range(NT):
                nc.sync.dma_start(adT[i][:, t * P:(t + 1) * P],
                                  d_cdT[i, t * 128:(t + 1) * 128, :])
            ps_kv = pp.tile([128, 1024], F32, tag="kv")
            for t in range(NT):
                xt = xs[i][:, t * 128:(t + 1) * 128]
                nc.tensor.matmul(ps_kv[:, t * 256:(t + 1) * 256],
                                 lhsT=xt, rhs=t_dkv, start=True, stop=True)
            kv3 = ps_kv.rearrange("p (t c) -> p t c", c=256)
            eks[i] = pers.tile([128, 512], BF, tag=f"ek{i}", name=f"ek{i}")
            ek3 = eks[i].rearrange("p (t c) -> p t c", c=E)
            nc.scalar.activation(ek3, kv3[:, :, 0:E], AF.Exp)
            ekvs[i] = pers.tile([128, 512], BF, tag=f"ekv{i}", name=f"ekv{i}")
            # fold sigmoid 0.5 and 1/sqrt(E) of the score into ekv
            nc.vector.scalar_tensor_tensor(
                ekvs[i].rearrange("p (t c) -> p t c", c=E),
                in0=kv3[:, :, E:2 * E], scalar=0.5 / SQRT_E, in1=ek3,
                op0=ALU.mult, op1=ALU.mult,
            )
            cap_t = wk2.tile([1, P], F32R, tag="cap")
            nc.sync.dma_start(cap_t, d_cap[i:i + 1, :])
            ps_qd = pq.tile([128, 512], F32, tag="q")
            nc.tensor.matmul(ps_qd, lhsT=t_dwql, rhs=cap_t,
                             start=True, stop=True)
            thd[i] = pers.tile([128, 512], BF, tag=f"th{i}", name=f"thd{i}")
            if dec_q_rank1:
                nc.scalar.activation(thd[i], ps_qd, AF.Tanh, scale=0.5)
            else:
                gsum = wk2.tile([128, 1], BF, tag="gsum")
                nc.vector.tensor_reduce(gsum, xs[i], axis=AX.X, op=ALU.add)
                ps_g = pn.tile([128, 512], F32, tag="nd")
                nc.tensor.matmul(ps_g[:, 0:1], lhsT=t_dwqm, rhs=gsum,
                                 start=True, stop=True)
                gqh = wk2.tile([128, 1], F32, tag="gqh")
                nc.vector.tensor_scalar_mul(gqh, ps_g[:, 0:1], 0.5 / N)
                nc.scalar.activation(thd[i], ps_qd, AF.Tanh, scale=0.5,
                                     bias=gqh[:, 0:1])

        for i in range(BI):
            # dB: dec num/den + aft (den first: rden is the DVE gate)
            ps_den = pn.tile([128, 512], F32, tag="nd")
            ps_num = pn.tile([128, 512], F32, tag="nd")
            for t in range(NT):
                nc.tensor.matmul(
                    ps_den, lhsT=eks[i][:, t * 128:(t + 1) * 128],
                    rhs=adT[i][:, t * 512:(t + 1) * 512],
                    start=(t == 0), stop=(t == NT - 1),
                )
            for t in range(NT):
                nc.tensor.matmul(
                    ps_num, lhsT=ekvs[i][:, t * 128:(t + 1) * 128],
                    rhs=adT[i][:, t * 512:(t + 1) * 512],
                    start=(t == 0), stop=(t == NT - 1),
                )
            rden = wk3.tile([128, 512], F32, tag="rden")
            nc.vector.reciprocal_approx_fast(rden, ps_den)
            w_ = wk2.tile([128, 512], BF, tag="w")
            nc.vector.tensor_tensor(w_, ps_num, rden, op=ALU.mult)
            aftd[i] = pers.tile([128, 512], BF, tag=f"pre{i}", name=f"aftd{i}")
            nc.vector.scalar_tensor_tensor(
                aftd[i], in0=thd[i], scalar=1.0, in1=w_,
                op0=ALU.add, op1=ALU.mult,
            )

            # dC: score + softmax (cur_dist pre-scaled by s_probs on host;
            # added into the score PSUM via an identity-matmul accumulate)
            if use_ninf:
                nstg = wk2.tile([128, PT * N], F32, tag="nstage")
                nc.sync.dma_start(
                    nstg.rearrange("p (t d) -> p t d", d=N),
                    d_ninf[i].rearrange("(t p) d -> p t d", p=128),
                )
            for pt in range(PT):
                cdc = wk4.tile([128, N], BF, tag="cdc")
                nc.sync.dma_start(cdc, d_cd[i, pt * 128:(pt + 1) * 128, :])
                ps_s = pn.tile([128, 512], F32, tag="nd")
                nc.tensor.matmul(ps_s, lhsT=aftd[i][:, pt * 128:(pt + 1) * 128],
                                 rhs=xs[i], start=True, stop=False)
                nc.tensor.matmul(ps_s, lhsT=t_ident, rhs=cdc,
                                 start=False, stop=True)
                th2 = wk6.tile([128, 512], F32, tag="th2")
                nc.scalar.activation(th2, ps_s, AF.Tanh)
                es = wk4.tile([128, 512], F32, tag="es")
                ssum = wk2.tile([128, 1], F32, tag="ssum")
                if use_ninf:
                    thm = wk2.tile([128, 512], F32, tag="thm")
                    nc.vector.scalar_tensor_tensor(
                        thm, in0=th2, scalar=CLIP,
                        in1=nstg[:, pt * N:(pt + 1) * N],
                        op0=ALU.mult, op1=ALU.add,
                    )
                    nc.scalar.activation(es, thm, AF.Exp, accum_out=ssum)
                else:
                    nc.scalar.activation(es, th2, AF.Exp, scale=CLIP,
                                         accum_out=ssum)
                o = wk4.tile([128, 512], F32, tag="o")
                nc.gpsimd.normalize_recip(o, es, ssum[:, 0:1])
                nc.sync.dma_start(d_out[i, pt * 128:(pt + 1) * 128, :], o)

    return nc


def _prep_maps(inputs):
    f32 = lambda a: np.ascontiguousarray(np.asarray(a, dtype=np.float32))
    bf = lambda a: np.ascontiguousarray(
        np.asarray(a, dtype=np.float32).astype(ml_dtypes.bfloat16))
    data = f32(inputs["data"])
    dist = f32(inputs["dist"])
    cur_dist = f32(inputs["cur_dist"])
    capacity = f32(inputs["capacity"])
    ninf = f32(inputs["ninf_mask"])
    meta = {
        "ls": float(np.asarray(inputs["log_scale"]).reshape(-1)[0]),
        "aft_alpha": [float(v) for v in np.asarray(inputs["aft_alpha"])],
        "dec_aft_alpha": float(np.asarray(inputs["dec_aft_alpha"]).reshape(-1)[0]),
        "probs_alpha": float(np.asarray(inputs["probs_alpha"]).reshape(-1)[0]),
        "use_ninf": bool(np.any(ninf)),
        "n1w_ones": bool(np.all(inputs["n1_w"] == 1.0)),
        "n1b_zero": not bool(np.any(inputs["n1_b"])),
        "n2w_ones": bool(np.all(inputs["n2_w"] == 1.0)),
        "n2b_zero": not bool(np.any(inputs["n2_b"])),
        "b1_zero": not bool(np.any(inputs["ff_b1"])),
        "b2_zero": not bool(np.any(inputs["ff_b2"])),
        "embb_zero": not bool(np.any(inputs["emb_b"])),
    }
    wq = f32(inputs["Wq"]).transpose(1, 0, 2).reshape(E, L * E)
    wkv = np.concatenate([f32(inputs["Wk"]), f32(inputs["Wv"])], axis=2)
    wkv = wkv.transpose(1, 0, 2).reshape(E, L * 2 * E)
    dkv = np.concatenate([f32(inputs["dWk"]), f32(inputs["dWv"])], axis=1)
    w1 = f32(inputs["ff_W1"]).transpose(1, 0, 2).reshape(E, L * F)
    w2 = (f32(inputs["ff_W2"]).reshape(L, FT, 128, E)
          .transpose(2, 0, 1, 3).reshape(128, L * FT * E))
    dwq = f32(inputs["dWq"])
    shared = {
        "emb_w": f32(inputs["emb_W"]),
        "wq_p": bf(wq), "wkv_p": bf(wkv),
        "w1_p": bf(w1), "w2_p": bf(w2),
        "dwq_main": bf(dwq[:E]),
        "ident": np.eye(128, dtype=np.float32).astype(ml_dtypes.bfloat16),
        "dwq_last": f32(dwq[E:E + 1]),
        "dkv": bf(dkv),
    }
    if not meta["embb_zero"]:
        shared["emb_b"] = f32(inputs["emb_b"]).reshape(E, 1)
    if not meta["n1w_ones"]:
        shared["n1wp"] = np.ascontiguousarray(f32(inputs["n1_w"]).T)
    if not meta["n1b_zero"]:
        shared["n1bp"] = np.ascontiguousarray(f32(inputs["n1_b"]).T)
    if not meta["n2w_ones"]:
        shared["n2wp"] = np.ascontiguousarray(f32(inputs["n2_w"]).T)
    if not meta["n2b_zero"]:
        shared["n2bp"] = np.ascontiguousarray(f32(inputs["n2_b"]).T)
    if not meta["b1_zero"]:
        shared["b1p"] = np.ascontiguousarray(
            f32(inputs["ff_b1"]).reshape(L, FT, 128).transpose(2, 0, 1)
            .reshape(128, L * FT))
    if not meta["b2_zero"]:
        shared["b2p"] = np.ascontiguousarray(f32(inputs["ff_b2"]).T)
    in_maps = []
    for c in range(NCORES):
        s = slice(c * BI, (c + 1) * BI)
        m = dict(shared)
        m["data_t"] = np.ascontiguousarray(data[s].transpose(0, 2, 1))
        s_enc0 = meta["ls"] * meta["aft_alpha"][0]
        s_dec = meta["ls"] * meta["dec_aft_alpha"]
        m["dist_t"] = bf(np.exp(s_enc0 * dist[s].transpose(0, 2, 1)))
        m["cur_dist"] = bf(meta["ls"] * meta["probs_alpha"] * cur_dist[s])
        m["cur_dist_t"] = bf(np.exp(
            s_dec * cur_dist[s].transpose(0, 2, 1)
            + ninf[s].transpose(0, 2, 1)))
        m["capacity"] = capacity[s]
        if meta["use_ninf"]:
            m["ninf"] = ninf[s]
        in_maps.append(m)
    return in_maps, meta


def kernel(_trace=False, **inputs):
    in_maps, meta = _prep_maps(inputs)
    nc = bacc.Bacc(None)
    _build(nc, meta)
    nc.finalize()
    res = run_bass_kernel_spmd(nc, in_maps, list(range(NCORES)), trace=_trace)
    out = np.concatenate([res.results[c]["out"] for c in range(NCORES)], axis=0)
    if _trace:
        return out.astype(np.float32), res
    return out.astype(np.float32)

